# revision 1
# baseline (speedup 1.0000x reference)
"""ExpertGNN (2x GCN + GAT + pool + fc) on 8 trn2 cores.

Sharding: dst-node blocks of 128 slots, balanced by in-degree via a node
permutation; 20 blocks per core. Edge-parallel gathers via indirect DMA;
segment sums via one-hot matmuls on the tensor engine; AllGather of node
tables between layers; AllReduce of the pooled vector at the end.
"""

import numpy as np

import concourse.bass as bass
import concourse.bacc as bacc
import concourse.mybir as mybir
import concourse.tile as tile

F32 = mybir.dt.float32
I32 = mybir.dt.int32
AX = mybir.AxisListType
ALU = mybir.AluOpType
ACT = mybir.ActivationFunctionType

NEG_SLOPE = 0.2
EPS = 1e-16


# ---------------------------------------------------------------- host prep

def prep(x, edge_index, n_cores=8):
    """Balance nodes into (n_cores * nblk) blocks of 128 slots, build padded
    per-core edge arrays (dst-block major)."""
    n = x.shape[0]
    src0 = np.asarray(edge_index[0], dtype=np.int64)
    dst0 = np.asarray(edge_index[1], dtype=np.int64)
    # self loops handled analytically on-device (no gather/one-hot needed)
    src_a = src0
    dst_a = dst0
    indeg = np.bincount(dst_a, minlength=n).astype(np.int64) + 1

    nblk_total = n_cores * int(np.ceil(n / 128.0 / n_cores))
    while nblk_total * 128 < n:
        nblk_total += n_cores
    nblk = nblk_total // n_cores

    # greedy balance by in-degree
    import heapq
    heap = [(0, 0, b) for b in range(nblk_total)]
    heapq.heapify(heap)
    order = np.argsort(-indeg, kind="stable")
    slot_of_node = np.empty(n, dtype=np.int64)
    block_nodes = [[] for _ in range(nblk_total)]
    for nd in order:
        load, cnt, b = heapq.heappop(heap)
        slot_of_node[nd] = b * 128 + cnt
        block_nodes[b].append(nd)
        if cnt + 1 < 128:
            heapq.heappush(heap, (load + int(indeg[nd]), cnt + 1, b))

    s_slot = slot_of_node[src_a]
    d_slot = slot_of_node[dst_a]
    d_blk = d_slot >> 7

    order_e = np.lexsort((s_slot, d_blk))
    s_slot = s_slot[order_e]
    d_slot = d_slot[order_e]
    d_blk = d_blk[order_e]
    starts = np.searchsorted(d_blk, np.arange(nblk_total))
    ends = np.searchsorted(d_blk, np.arange(nblk_total), side="right")
    counts = ends - starts
    tpb = int(np.ceil(counts.max() / 128.0))
    cap = tpb * 128

    # per-block padded arrays, [128, tpb] layout: edge e -> [e%128, e//128]
    gidx = np.zeros((nblk_total, 128, tpb), dtype=np.int32)
    dstv = np.full((nblk_total, 128, tpb), -1.0, dtype=np.float32)
    ee = np.arange(cap)
    for b in range(nblk_total):
        cnt = counts[b]
        gs = np.zeros(cap, dtype=np.int32)
        gd = np.full(cap, -1.0, dtype=np.float32)
        gs[:cnt] = s_slot[starts[b]:ends[b]]
        gd[:cnt] = (d_slot[starts[b]:ends[b]] & 127).astype(np.float32)
        gidx[b, ee % 128, ee // 128] = gs
        dstv[b, ee % 128, ee // 128] = gd

    # per-core node data
    xc = np.zeros((nblk_total, 128, x.shape[1]), dtype=np.float32)
    vmask = np.zeros((nblk_total, 128, 1), dtype=np.float32)
    for b in range(nblk_total):
        nds = block_nodes[b]
        if nds:
            xc[b, :len(nds), :] = x[nds]
            vmask[b, :len(nds), 0] = 1.0

    meta = dict(nblk=nblk, tpb=tpb, nblk_total=nblk_total,
                nslots=nblk_total * 128, n=n)
    per_core = []
    for c in range(n_cores):
        sl = slice(c * nblk, (c + 1) * nblk)
        per_core.append(dict(
            gidx=gidx[sl].copy(),
            dstv=dstv[sl].copy(),
            xc=xc[sl].copy(),
            vmask=vmask[sl].copy(),
        ))
    return per_core, meta


# ------------------------------------------------------------ device program

def build(meta, weights_shapes, n_cores=8, n_queues=2,
          timing_repeats=0, scratch=32768, src_bufs=6):
    nblk = meta["nblk"]
    tpb = meta["tpb"]
    nslots = meta["nslots"]
    n_real = meta["n"]
    f_in = weights_shapes["f_in"]     # 128
    d1 = weights_shapes["d1"]         # 64
    d2 = weights_shapes["d2"]         # 128
    h_heads = weights_shapes["h"]     # 8
    f_gat = weights_shapes["f"]       # 128
    hf = h_heads * f_gat              # 1024
    ncls = weights_shapes["ncls"]     # 10
    own = nblk * 128

    nc = bacc.Bacc("TRN2", target_bir_lowering=False, debug=False,
                   num_devices=n_cores, num_swdge_queues=max(1, n_queues),
                   dynamic_dma_scratch_size=scratch)

    def inp(name, shape, dt=F32):
        return nc.dram_tensor(name, shape, dt, kind="ExternalInput")

    gidx = inp("gidx", [nblk, 128, tpb], I32)
    dstv = inp("dstv", [nblk, 128, tpb])
    xc = inp("xc", [nblk, 128, f_in])
    vmask = inp("vmask", [nblk, 128, 1])
    w1 = inp("w1", [f_in, d1])
    b1 = inp("b1", [128, d1])
    w2 = inp("w2", [d1, d2])
    b2 = inp("b2", [128, d2])
    wg = inp("wg", [d2, hf])
    asrc = inp("asrc", [h_heads, f_gat])
    adst = inp("adst", [h_heads, f_gat])
    bg = inp("bg", [128, hf])
    wfc = inp("wfc", [hf, ncls])
    bfc = inp("bfc", [ncls, 1])
    out = nc.dram_tensor("out", [ncls], F32, kind="ExternalOutput")

    # internal shared DRAM for collectives / gather tables
    def shared(name, shape):
        return nc.dram_tensor(name, shape, F32, kind="Internal",
                              addr_space="Shared")

    def local(name, shape):
        return nc.dram_tensor(name, shape, F32, kind="Internal")

    xs_own = local("xs_own", [own, f_in])
    xs_full = shared("xs_full", [nslots, f_in])
    t1_own = local("t1_own", [own, d1])
    t1_full = shared("t1_full", [nslots, d1])
    t2_own = local("t2_own", [own, d2 + h_heads])
    t2_full = shared("t2_full", [nslots, d2 + h_heads])
    pool_in = local("pool_in", [1, hf])
    pool_out = shared("pool_out", [1, hf])
    ald_dram = nc.dram_tensor("ald_dram", [nblk, 128, h_heads], F32,
                              kind="Internal")
    scratch = nc.dram_tensor("scratch", [1, max(hf, 32)], F32, kind="Internal")

    d2h = d2 + h_heads
    rg = [list(range(n_cores))]
    q_counter = [0]

    def gather(dst_tile_ap, table_ap, idx_ap):
        inst = nc.gpsimd.indirect_dma_start(
            out=dst_tile_ap, out_offset=None, in_=table_ap,
            in_offset=bass.IndirectOffsetOnAxis(ap=idx_ap, axis=0))
        if n_queues > 1:
            qi = q_counter[0] % n_queues
            q_counter[0] += 1
            inst.ins.queue = f"qPoolDynamic{qi or ''}"
        return inst

    with tile.TileContext(nc) as tc:
        with tc.tile_pool(name="const", bufs=1) as constp, \
             tc.tile_pool(name="meta", bufs=4) as metap, \
             tc.tile_pool(name="src", bufs=src_bufs) as srcp, \
             tc.tile_pool(name="oh", bufs=4) as ohp, \
             tc.tile_pool(name="blk", bufs=2) as blkp, \
             tc.tile_pool(name="srcblk", bufs=2) as srcblkp, \
             tc.tile_pool(name="exblk", bufs=2) as exblkp, \
             tc.tile_pool(name="small", bufs=4) as smallp, \
             tc.tile_pool(name="dinvp", bufs=1) as dinvp, \
             tc.tile_pool(name="psAcc", bufs=2, space="PSUM") as psAcc, \
             tc.tile_pool(name="psBig", bufs=1, space="PSUM") as psBig, \
             tc.tile_pool(name="psTr", bufs=2, space="PSUM") as psTr:

            # ---------------- constants
            iota_row = constp.tile([128, 128], F32)  # [p, j] = j
            nc.gpsimd.iota(iota_row[:], pattern=[[1, 128]], base=0,
                           channel_multiplier=0,
                           allow_small_or_imprecise_dtypes=True)
            iota_col = constp.tile([128, 128], F32)  # [p, j] = p
            nc.gpsimd.iota(iota_col[:], pattern=[[0, 128]], base=0,
                           channel_multiplier=1,
                           allow_small_or_imprecise_dtypes=True)
            ones_col = constp.tile([128, 1], F32)
            nc.vector.memset(ones_col[:], 1.0)
            from concourse.masks import make_identity
            ident = constp.tile([128, 128], F32)
            make_identity(nc, ident[:])

            w1sb = constp.tile([f_in, d1], F32)
            nc.sync.dma_start(out=w1sb[:], in_=w1[:])
            b1sb = constp.tile([128, d1], F32)
            nc.sync.dma_start(out=b1sb[:], in_=b1[:])
            w2sb = constp.tile([d1, d2], F32)
            nc.sync.dma_start(out=w2sb[:], in_=w2[:])
            b2sb = constp.tile([128, d2], F32)
            nc.sync.dma_start(out=b2sb[:], in_=b2[:])
            wgsb = constp.tile([d2, hf], F32)
            nc.sync.dma_start(out=wgsb[:], in_=wg[:])
            bgsb = constp.tile([128, hf], F32)
            nc.sync.dma_start(out=bgsb[:], in_=bg[:])
            asrcsb = constp.tile([h_heads, f_gat], F32)
            nc.sync.dma_start(out=asrcsb[:], in_=asrc[:])
            adstsb = constp.tile([h_heads, f_gat], F32)
            nc.sync.dma_start(out=adstsb[:], in_=adst[:])

            # Aw[k,h] = sum_f Wg[k, h*F+f] * a_src[h, f]  (and Ad likewise)
            asrcT_ps = psTr.tile([f_gat, h_heads], F32, space="PSUM", tag="tr")
            nc.tensor.transpose(out=asrcT_ps[:], in_=asrcsb[:],
                                identity=ident[:h_heads, :h_heads])
            asrcT = constp.tile([f_gat, h_heads], F32)
            nc.vector.tensor_copy(out=asrcT[:], in_=asrcT_ps[:])
            adstT_ps = psTr.tile([f_gat, h_heads], F32, space="PSUM", tag="tr")
            nc.tensor.transpose(out=adstT_ps[:], in_=adstsb[:],
                                identity=ident[:h_heads, :h_heads])
            adstT = constp.tile([f_gat, h_heads], F32)
            nc.vector.tensor_copy(out=adstT[:], in_=adstT_ps[:])
            aw = constp.tile([d2, h_heads], F32)
            ad = constp.tile([d2, h_heads], F32)
            for hh in range(h_heads):
                wgT_ps = psTr.tile([f_gat, d2], F32, space="PSUM", tag="tr")
                nc.tensor.transpose(
                    out=wgT_ps[:],
                    in_=wgsb[:, hh * f_gat:(hh + 1) * f_gat],
                    identity=ident[:])
                wgT = blkp.tile([f_gat, d2], F32, tag="wgT")
                nc.vector.tensor_copy(out=wgT[:], in_=wgT_ps[:])
                aw_ps = psTr.tile([d2, 1], F32, space="PSUM", tag="tr")
                nc.tensor.matmul(out=aw_ps[:], lhsT=wgT[:],
                                 rhs=asrcT[:, hh:hh + 1], start=True, stop=True)
                nc.vector.tensor_copy(out=aw[:, hh:hh + 1], in_=aw_ps[:])
                ad_ps = psTr.tile([d2, 1], F32, space="PSUM", tag="tr")
                nc.tensor.matmul(out=ad_ps[:], lhsT=wgT[:],
                                 rhs=adstT[:, hh:hh + 1], start=True, stop=True)
                nc.vector.tensor_copy(out=ad[:, hh:hh + 1], in_=ad_ps[:])

            def _whole_body(_i=None):
                dinv_all = dinvp.tile([128, nblk], F32)  # per-block dinv columns

                # ---------------- phase 0: degree -> dinv, xs table
                for b in range(nblk):
                    dv = metap.tile([128, tpb], F32, tag="dstv")
                    nc.sync.dma_start(out=dv[:], in_=dstv[b])
                    deg_ps = psAcc.tile([128, 1], F32, space="PSUM", tag="acc")
                    for t in range(tpb):
                        oh = ohp.tile([128, 128], F32, tag="oh0")
                        nc.vector.tensor_tensor(
                            out=oh[:], in0=dv[:, t:t + 1].to_broadcast([128, 128]),
                            in1=iota_row[:], op=ALU.is_equal)
                        nc.tensor.matmul(out=deg_ps[:], lhsT=oh[:], rhs=ones_col[:],
                                         start=(t == 0), stop=(t == tpb - 1))
                    deg = smallp.tile([128, 1], F32, tag="deg_s")
                    nc.vector.tensor_scalar(out=deg[:], in0=deg_ps[:], scalar1=1.0,
                                            scalar2=None, op0=ALU.add)
                    nc.vector.reciprocal(out=deg[:], in_=deg[:])
                    nc.scalar.activation(out=dinv_all[:, b:b + 1], in_=deg[:],
                                         func=ACT.Sqrt)
                    xb = blkp.tile([128, f_in], F32, tag="xb")
                    nc.sync.dma_start(out=xb[:], in_=xc[b])
                    xs_blk = blkp.tile([128, f_in], F32, tag="xsb")
                    nc.vector.tensor_tensor(
                        out=xs_blk[:], in0=xb[:],
                        in1=dinv_all[:, b:b + 1].to_broadcast([128, f_in]),
                        op=ALU.mult)
                    nc.sync.dma_start(out=xs_own[b * 128:(b + 1) * 128, :],
                                      in_=xs_blk[:])

                if timing_repeats:
                    nc.sync.dma_start(out=xs_full[:own, :], in_=xs_own[:])
                else:
                    nc.gpsimd.collective_compute(
                        "AllGather", ALU.bypass, replica_groups=rg,
                        ins=[xs_own[:]], outs=[xs_full[:]])

                # ---------------- phase 1: GCN layer 1 -> t1 table
                for b in range(nblk):
                    gi = metap.tile([128, tpb], I32, tag="gidx")
                    nc.sync.dma_start(out=gi[:], in_=gidx[b])
                    dv = metap.tile([128, tpb], F32, tag="dstv")
                    nc.sync.dma_start(out=dv[:], in_=dstv[b])
                    aggT = psAcc.tile([128, 128], F32, space="PSUM", tag="acc")
                    for t in range(tpb):
                        srct = srcp.tile([128, f_in], F32, tag="src1")
                        gather(srct[:], xs_full[:], gi[:, t:t + 1])
                        oh = ohp.tile([128, 128], F32, tag="oh1")
                        nc.vector.tensor_tensor(
                            out=oh[:], in0=dv[:, t:t + 1].to_broadcast([128, 128]),
                            in1=iota_row[:], op=ALU.is_equal)
                        nc.tensor.matmul(out=aggT[:], lhsT=srct[:], rhs=oh[:],
                                         start=(t == 0), stop=False)
                    xsb_r = blkp.tile([128, f_in], F32, tag="xsbr")
                    nc.sync.dma_start(out=xsb_r[:],
                                      in_=xs_own[b * 128:(b + 1) * 128, :])
                    nc.tensor.matmul(out=aggT[:], lhsT=xsb_r[:], rhs=ident[:],
                                     start=False, stop=True)
                    # h1 = relu(dinv * (aggT.T @ W1) + b1); t1 = dinv * h1
                    aggs = blkp.tile([128, 128], F32, tag="agg1s")
                    nc.vector.tensor_copy(out=aggs[:], in_=aggT[:])
                    h1ps = psTr.tile([128, d1], F32, space="PSUM", tag="tr")
                    nc.tensor.matmul(out=h1ps[:], lhsT=aggs[:], rhs=w1sb[:],
                                     start=True, stop=True)
                    h1a = blkp.tile([128, d1], F32, tag="h1a")
                    nc.vector.tensor_tensor(
                        out=h1a[:], in0=h1ps[:],
                        in1=dinv_all[:, b:b + 1].to_broadcast([128, d1]),
                        op=ALU.mult)
                    nc.vector.tensor_tensor(
                        out=h1a[:], in0=h1a[:],
                        in1=b1sb[:], op=ALU.add)
                    nc.scalar.activation(out=h1a[:], in_=h1a[:], func=ACT.Relu)
                    nc.vector.tensor_tensor(
                        out=h1a[:], in0=h1a[:],
                        in1=dinv_all[:, b:b + 1].to_broadcast([128, d1]),
                        op=ALU.mult)
                    nc.sync.dma_start(out=t1_own[b * 128:(b + 1) * 128, :],
                                      in_=h1a[:])

                if timing_repeats:
                    nc.sync.dma_start(out=t1_full[:own, :], in_=t1_own[:])
                else:
                    nc.gpsimd.collective_compute(
                        "AllGather", ALU.bypass, replica_groups=rg,
                        ins=[t1_own[:]], outs=[t1_full[:]])

                # ---------------- phase 2: GCN layer 2 -> t2 table [h2 | als]
                for b in range(nblk):
                    gi = metap.tile([128, tpb], I32, tag="gidx")
                    nc.sync.dma_start(out=gi[:], in_=gidx[b])
                    dv = metap.tile([128, tpb], F32, tag="dstv")
                    nc.sync.dma_start(out=dv[:], in_=dstv[b])
                    aggT = psAcc.tile([d1, 128], F32, space="PSUM", tag="acc")
                    for t in range(tpb):
                        srct = srcp.tile([128, d1], F32, tag="src2")
                        gather(srct[:], t1_full[:], gi[:, t:t + 1])
                        oh = ohp.tile([128, 128], F32, tag="oh2")
                        nc.vector.tensor_tensor(
                            out=oh[:], in0=dv[:, t:t + 1].to_broadcast([128, 128]),
                            in1=iota_row[:], op=ALU.is_equal)
                        nc.tensor.matmul(out=aggT[:], lhsT=srct[:], rhs=oh[:],
                                         start=(t == 0), stop=False)
                    t1b_r = blkp.tile([128, d1], F32, tag="t1br")
                    nc.sync.dma_start(out=t1b_r[:],
                                      in_=t1_own[b * 128:(b + 1) * 128, :])
                    nc.tensor.matmul(out=aggT[:], lhsT=t1b_r[:],
                                     rhs=ident[:, :128], start=False, stop=True)
                    aggs = blkp.tile([d1, 128], F32, tag="agg2s")
                    nc.vector.tensor_copy(out=aggs[:], in_=aggT[:])
                    h2ps = psTr.tile([128, d2], F32, space="PSUM", tag="tr")
                    nc.tensor.matmul(out=h2ps[:], lhsT=aggs[:], rhs=w2sb[:],
                                     start=True, stop=True)
                    h2t = blkp.tile([128, d2h], F32, tag="h2t")
                    nc.vector.tensor_tensor(
                        out=h2t[:, :d2], in0=h2ps[:],
                        in1=dinv_all[:, b:b + 1].to_broadcast([128, d2]),
                        op=ALU.mult)
                    nc.vector.tensor_tensor(
                        out=h2t[:, :d2], in0=h2t[:, :d2],
                        in1=b2sb[:], op=ALU.add)
                    nc.scalar.activation(out=h2t[:, :d2], in_=h2t[:, :d2],
                                         func=ACT.Relu)
                    # als/ald: need h2^T
                    h2T_ps = psTr.tile([d2, 128], F32, space="PSUM", tag="tr")
                    nc.tensor.transpose(out=h2T_ps[:], in_=h2t[:, :d2],
                                        identity=ident[:])
                    h2T = blkp.tile([d2, 128], F32, tag="h2Ts")
                    nc.vector.tensor_copy(out=h2T[:], in_=h2T_ps[:])
                    als_ps = psTr.tile([128, h_heads], F32, space="PSUM", tag="tr")
                    nc.tensor.matmul(out=als_ps[:], lhsT=h2T[:], rhs=aw[:],
                                     start=True, stop=True)
                    nc.vector.tensor_copy(out=h2t[:, d2:], in_=als_ps[:])
                    ald_ps = psTr.tile([128, h_heads], F32, space="PSUM", tag="tr")
                    nc.tensor.matmul(out=ald_ps[:], lhsT=h2T[:], rhs=ad[:],
                                     start=True, stop=True)
                    aldsb = smallp.tile([128, h_heads], F32, tag="aldsb")
                    nc.vector.tensor_copy(out=aldsb[:], in_=ald_ps[:])
                    nc.sync.dma_start(out=ald_dram[b], in_=aldsb[:])
                    nc.sync.dma_start(out=t2_own[b * 128:(b + 1) * 128, :],
                                      in_=h2t[:])

                if timing_repeats:
                    nc.sync.dma_start(out=t2_full[:own, :], in_=t2_own[:])
                else:
                    nc.gpsimd.collective_compute(
                        "AllGather", ALU.bypass, replica_groups=rg,
                        ins=[t2_own[:]], outs=[t2_full[:]])

                # ---------------- phase 3: GAT + pooled partial
                pooled = dinvp.tile([1, hf], F32)
                nc.vector.memset(pooled[:], 0.0)
                for b in range(nblk):
                    gi = metap.tile([128, tpb], I32, tag="gidx")
                    nc.sync.dma_start(out=gi[:], in_=gidx[b])
                    dv = metap.tile([128, tpb], F32, tag="dstv")
                    nc.sync.dma_start(out=dv[:], in_=dstv[b])
                    aldb = smallp.tile([128, h_heads], F32, tag="aldb")
                    nc.sync.dma_start(out=aldb[:], in_=ald_dram[b])
                    srcts = srcblkp.tile([128, tpb, d2h], F32, tag="srcts")
                    exts = exblkp.tile([128, tpb, h_heads], F32, tag="exts")
                    s_ps = psAcc.tile([128, h_heads], F32, space="PSUM", tag="acc")
                    # pass A
                    for t in range(tpb):
                        gather(srcts[:, t, :], t2_full[:], gi[:, t:t + 1])
                        oh = ohp.tile([128, 128], F32, tag="oh3")
                        nc.vector.tensor_tensor(
                            out=oh[:], in0=dv[:, t:t + 1].to_broadcast([128, 128]),
                            in1=iota_row[:], op=ALU.is_equal)
                        ohT_ps = psTr.tile([128, 128], F32, space="PSUM", tag="tr")
                        nc.tensor.transpose(out=ohT_ps[:], in_=oh[:],
                                            identity=ident[:])
                        ohT = ohp.tile([128, 128], F32, tag="ohT_s")
                        nc.vector.tensor_copy(out=ohT[:], in_=ohT_ps[:])
                        alde_ps = psTr.tile([128, h_heads], F32, space="PSUM",
                                           tag="tr")
                        nc.tensor.matmul(out=alde_ps[:], lhsT=ohT[:], rhs=aldb[:],
                                         start=True, stop=True)
                        lg = smallp.tile([128, h_heads], F32, tag="lg")
                        nc.vector.tensor_tensor(out=lg[:], in0=srcts[:, t, d2:],
                                                in1=alde_ps[:], op=ALU.add)
                        lneg = smallp.tile([128, h_heads], F32, tag="lneg")
                        nc.vector.tensor_scalar(out=lneg[:], in0=lg[:],
                                                scalar1=0.0, scalar2=NEG_SLOPE,
                                                op0=ALU.min, op1=ALU.mult)
                        nc.vector.tensor_scalar(out=lg[:], in0=lg[:], scalar1=0.0,
                                                scalar2=None, op0=ALU.max)
                        nc.vector.tensor_tensor(out=lg[:], in0=lg[:], in1=lneg[:],
                                                op=ALU.add)
                        nc.scalar.activation(out=exts[:, t, :], in_=lg[:],
                                             func=ACT.Exp)
                        nc.tensor.matmul(out=s_ps[:], lhsT=oh[:],
                                         rhs=exts[:, t, :],
                                         start=(t == 0), stop=False)
                    h2b_r = srcblkp.tile([128, d2h], F32, tag="h2br")
                    nc.sync.dma_start(out=h2b_r[:],
                                      in_=t2_own[b * 128:(b + 1) * 128, :])
                    lgs = smallp.tile([128, h_heads], F32, tag="lgs")
                    nc.vector.tensor_tensor(out=lgs[:], in0=h2b_r[:, d2:],
                                            in1=aldb[:], op=ALU.add)
                    lnegs = smallp.tile([128, h_heads], F32, tag="lnegs")
                    nc.vector.tensor_scalar(out=lnegs[:], in0=lgs[:],
                                            scalar1=0.0, scalar2=NEG_SLOPE,
                                            op0=ALU.min, op1=ALU.mult)
                    nc.vector.tensor_scalar(out=lgs[:], in0=lgs[:],
                                            scalar1=0.0, scalar2=None,
                                            op0=ALU.max)
                    nc.vector.tensor_tensor(out=lgs[:], in0=lgs[:],
                                            in1=lnegs[:], op=ALU.add)
                    ex_self = smallp.tile([128, h_heads], F32, tag="exself")
                    nc.scalar.activation(out=ex_self[:], in_=lgs[:],
                                         func=ACT.Exp)
                    nc.tensor.matmul(out=s_ps[:], lhsT=ident[:],
                                     rhs=ex_self[:], start=False, stop=True)
                    # r = 1/(s+eps), transposed to [h, slot] rows
                    rblk = smallp.tile([128, h_heads], F32, tag="rblk")
                    nc.vector.tensor_scalar(out=rblk[:], in0=s_ps[:], scalar1=EPS,
                                            scalar2=None, op0=ALU.add)
                    nc.vector.reciprocal(out=rblk[:], in_=rblk[:])
                    # pass B
                    aggT = psAcc.tile([128, h_heads, 128], F32, space="PSUM",
                                      tag="acc")
                    for t in range(tpb):
                        oh = ohp.tile([128, 128], F32, tag="oh3")
                        nc.vector.tensor_tensor(
                            out=oh[:], in0=dv[:, t:t + 1].to_broadcast([128, 128]),
                            in1=iota_row[:], op=ALU.is_equal)
                        ohT_ps = psTr.tile([128, 128], F32, space="PSUM", tag="tr")
                        nc.tensor.transpose(out=ohT_ps[:], in_=oh[:],
                                            identity=ident[:])
                        ohT = ohp.tile([128, 128], F32, tag="ohT_s")
                        nc.vector.tensor_copy(out=ohT[:], in_=ohT_ps[:])
                        re_ps = psTr.tile([128, h_heads], F32, space="PSUM",
                                         tag="tr")
                        nc.tensor.matmul(out=re_ps[:], lhsT=ohT[:], rhs=rblk[:],
                                         start=True, stop=True)
                        alpha = smallp.tile([128, h_heads], F32, tag="alpha")
                        nc.vector.tensor_tensor(out=alpha[:], in0=exts[:, t, :],
                                                in1=re_ps[:], op=ALU.mult)
                        ohex = ohp.tile([128, h_heads, 128], F32, tag="ohex")
                        nc.vector.tensor_tensor(
                            out=ohex[:, :, :],
                            in0=oh[:].rearrange("p (o j) -> p o j", o=1)
                                  .to_broadcast([128, h_heads, 128]),
                            in1=alpha[:].rearrange("p (h o) -> p h o", o=1)
                                  .to_broadcast([128, h_heads, 128]),
                            op=ALU.mult)
                        for hh in range(h_heads):
                            # one zero-region arm per 2KB bank (4 heads/bank)
                            nc.tensor.matmul(out=aggT[:, hh, :],
                                             lhsT=srcts[:, t, :d2],
                                             rhs=ohex[:, hh, :],
                                             start=(t == 0 and hh % 4 == 0),
                                             stop=False,
                                             skip_group_check=True)
                    alpha_s = smallp.tile([128, h_heads], F32, tag="alphas")
                    nc.vector.tensor_tensor(out=alpha_s[:], in0=ex_self[:],
                                            in1=rblk[:], op=ALU.mult)
                    for hh in range(h_heads):
                        hsc = ohp.tile([128, d2], F32, tag="hsc")
                        nc.vector.tensor_tensor(
                            out=hsc[:], in0=h2b_r[:, :d2],
                            in1=alpha_s[:, hh:hh + 1].to_broadcast([128, d2]),
                            op=ALU.mult)
                        nc.tensor.matmul(out=aggT[:, hh, :], lhsT=hsc[:],
                                         rhs=ident[:], start=False, stop=True,
                                         skip_group_check=True)
                    # out_gat[slot, h*F+f] = sum_k r-scaled aggT -> @ Wg_h
                    og_ps = psBig.tile([128, hf], F32, space="PSUM", tag="big")
                    aggs3 = blkp.tile([128, h_heads, 128], F32, tag="agg3s")
                    nc.vector.tensor_copy(out=aggs3[:, :, :], in_=aggT[:, :, :])
                    for hh in range(h_heads):
                        nc.tensor.matmul(
                            out=og_ps[:, hh * f_gat:(hh + 1) * f_gat],
                            lhsT=aggs3[:, hh, :],
                            rhs=wgsb[:, hh * f_gat:(hh + 1) * f_gat],
                            start=True, stop=True)
                    gat = blkp.tile([128, hf], F32, tag="gat")
                    nc.vector.tensor_tensor(
                        out=gat[:], in0=og_ps[:], in1=bgsb[:], op=ALU.add)
                    nc.scalar.activation(out=gat[:], in_=gat[:], func=ACT.Relu)
                    vm = smallp.tile([128, 1], F32, tag="vm")
                    nc.sync.dma_start(out=vm[:], in_=vmask[b])
                    for half in range(2):
                        pool_ps = psTr.tile([1, hf // 2], F32, space="PSUM",
                                           tag="tr")
                        nc.tensor.matmul(
                            out=pool_ps[:],
                            lhsT=vm[:],
                            rhs=gat[:, half * (hf // 2):(half + 1) * (hf // 2)],
                            start=True, stop=True)
                        nc.vector.tensor_tensor(
                            out=pooled[:1, half * (hf // 2):(half + 1) * (hf // 2)],
                            in0=pooled[:1, half * (hf // 2):(half + 1) * (hf // 2)],
                            in1=pool_ps[:1, :], op=ALU.add)

                # ---------------- phase 4: AllReduce pooled, fc, softmax
                nc.sync.dma_start(out=pool_in[:], in_=pooled[:1, :])
                if timing_repeats:
                    nc.sync.dma_start(out=pool_out[:], in_=pool_in[:])
                else:
                    nc.gpsimd.collective_compute(
                        "AllReduce", ALU.add, replica_groups=rg,
                        ins=[pool_in[:]], outs=[pool_out[:]])
                mean = smallp.tile([1, hf], F32, tag="mean")
                nc.sync.dma_start(out=mean[:], in_=pool_out[:])
                nc.vector.tensor_scalar(out=mean[:], in0=mean[:],
                                        scalar1=1.0 / n_real, scalar2=None,
                                        op0=ALU.mult)
                nc.sync.dma_start(out=scratch[0, :hf], in_=mean[:1, :])
                fc_ps = psAcc.tile([ncls, 1], F32, space="PSUM", tag="acc")
                n_chunks = hf // 128
                for ci in range(n_chunks):
                    mcol = smallp.tile([128, 1], F32, tag="mcol")
                    nc.sync.dma_start(out=mcol[:],
                                      in_=scratch[0, ci * 128:(ci + 1) * 128, None])
                    wfc_sb = smallp.tile([128, ncls], F32, tag="wfcsb")
                    nc.sync.dma_start(out=wfc_sb[:],
                                      in_=wfc[ci * 128:(ci + 1) * 128, :])
                    nc.tensor.matmul(out=fc_ps[:], lhsT=wfc_sb[:], rhs=mcol[:],
                                     start=(ci == 0), stop=(ci == n_chunks - 1))
                bfc_sb = smallp.tile([ncls, 1], F32, tag="bfcsb")
                nc.sync.dma_start(out=bfc_sb[:], in_=bfc[:])
                logit = smallp.tile([ncls, 1], F32, tag="logit")
                nc.vector.tensor_tensor(out=logit[:], in0=fc_ps[:], in1=bfc_sb[:],
                                        op=ALU.add)
                nc.sync.dma_start(out=scratch[0, :ncls, None], in_=logit[:, :1])
                lrow = smallp.tile([1, ncls], F32, tag="lrow")
                nc.sync.dma_start(out=lrow[:], in_=scratch[:1, :ncls])
                erow = smallp.tile([1, ncls], F32, tag="erow")
                nc.scalar.activation(out=erow[:], in_=lrow[:], func=ACT.Exp)
                ssum = smallp.tile([1, 1], F32, tag="ssum")
                nc.vector.reduce_sum(out=ssum[:], in_=erow[:], axis=AX.X)
                nc.vector.reciprocal(out=ssum[:], in_=ssum[:])
                nc.vector.tensor_tensor(
                    out=erow[:], in0=erow[:],
                    in1=ssum[:1, :1].to_broadcast([1, ncls]), op=ALU.mult)
                nc.sync.dma_start(out=out[None, :], in_=erow[:1, :])

            if timing_repeats:
                with tc.For_i(0, timing_repeats, 1) as _i:
                    _whole_body(_i)
            else:
                _whole_body()

    nc.compile()
    return nc


def make_in_maps(per_core, w):
    maps = []
    for pc in per_core:
        m = dict(pc)
        m.update(w)
        maps.append(m)
    return maps


def weights_dict(W1, b1, W2, b2, Wg, a_src, a_dst, bg, Wfc, bfc):
    return dict(
        w1=np.asarray(W1, np.float32),
        b1=np.tile(np.asarray(b1, np.float32).reshape(1, -1), (128, 1)),
        w2=np.asarray(W2, np.float32),
        b2=np.tile(np.asarray(b2, np.float32).reshape(1, -1), (128, 1)),
        wg=np.asarray(Wg, np.float32),
        asrc=np.asarray(a_src, np.float32),
        adst=np.asarray(a_dst, np.float32),
        bg=np.tile(np.asarray(bg, np.float32).reshape(1, -1), (128, 1)),
        wfc=np.asarray(Wfc, np.float32),
        bfc=np.asarray(bfc, np.float32).reshape(-1, 1),
    )


# ------------------------------------------------------------ harness entry

_CACHE = {}


def kernel(**inputs):
    """Full-input entry: shards across 8 trn2 cores internally."""
    x = np.asarray(inputs["x"], dtype=np.float32)
    edge_index = np.asarray(inputs["edge_index"])
    n_cores = 8

    per_core, meta = prep(x, edge_index, n_cores=n_cores)
    shapes = dict(f_in=128, d1=64, d2=128, h=8, f=128, ncls=10)

    key = (meta["nblk"], meta["tpb"], meta["nslots"], meta["n"])
    if key in _CACHE:
        nc = _CACHE[key]
    else:
        nc = build(meta, shapes, n_cores=n_cores, n_queues=4)
        _CACHE[key] = nc

    wd = weights_dict(inputs["W1"], inputs["b1"], inputs["W2"],
                      inputs["b2"], inputs["Wg"], inputs["a_src"],
                      inputs["a_dst"], inputs["bg"], inputs["Wfc"],
                      inputs["bfc"])
    in_maps = make_in_maps(per_core, wd)

    from concourse.bass_utils import run_bass_kernel_spmd
    res = run_bass_kernel_spmd(nc, in_maps, core_ids=list(range(n_cores)))
    return np.asarray(res.results[0]["out"], dtype=np.float32)

